# revision 29
# baseline (speedup 1.0000x reference)
"""Segment-masked attention kernel for Trainium2 (8 NeuronCores).

Problem: B=1, H=16, S=4096, D=128, NSEG=2 segment-id masked softmax attention.

Strategy (v2):
  * Host: stable-argsort q/kv positions by segment id -> two dense
    block-diagonal attentions (one per segment), ~half the FLOPs, no masks.
    Outputs are scattered back to original q order on host.
  * Shard: 2 heads per core across 8 cores (head-parallel, no comms).
  * Per (head, segment): q processed in blocks of <=512 columns (tail
    rebalanced so every block is >=256 wide); kv in chunks of 128 rows,
    chunks grouped in threes so each ACT(exp) instruction covers
    FD=3*512=1536 elements (amortizes the ~300-cycle ACT issue overhead).
  * Dtypes: scores matmul in fp32r (1 cycle/col on PE); exp output pt in
    bf16; PV matmul bf16xbf16 (1 cycle/col).
  * Softmax sums WITHOUT the per-chunk ones-matmul (which would cost as many
    PE column-cycles as the whole PV): DVE accumulates pt chunk-wise into a
    wide bf16 accumulator (tensor_add at 2x rate), collapses it at block end,
    and a single 1-row ones-matmul per block (N=W columns) finishes the
    partition-dim reduction.  PE sums cost drops ~16x.
  * Software pipeline depth 2 over (block, chunk-group) items keeps PE
    streaming while ACT computes exp: scores(i) || exp(i-1) || pv(i-2).
  * kv segments are padded to even length with zero rows whose exp(0)=1 is
    subtracted from the sums; no max-subtraction is needed (scaled scores are
    ~N(0,1), exp never overflows fp32, softmax is shift invariant).
"""

import os

import numpy as np

_PROGRAM_CACHE = {}
last_exec_time_ns = None


def _install_ntff_hook():
    """Provide antenv.axon_hooks (missing in this image) so that
    run_bass_kernel_spmd(trace=True) can capture an NTFF profile."""
    import contextlib
    import ctypes
    import sys
    import types

    try:
        from antenv.axon_hooks import get_axon_ntff_profile_hook  # noqa: F401

        return True  # real module exists
    except ImportError:
        pass

    so_path = "/opt/axon/libaxon_pjrt.so"
    if not os.path.exists(so_path):
        return False
    lib = ctypes.CDLL(so_path)
    if not hasattr(lib, "axon_start_nrt_profile"):
        return False
    lib.axon_start_nrt_profile.argtypes = [
        ctypes.POINTER(ctypes.c_int64),
        ctypes.c_size_t,
    ]
    lib.axon_start_nrt_profile.restype = ctypes.c_int64
    lib.axon_stop_nrt_profile.argtypes = [ctypes.c_char_p]
    lib.axon_stop_nrt_profile.restype = ctypes.c_int64

    @contextlib.contextmanager
    def _hook(output_dir, device_ids):
        import jax

        jax.devices()
        if device_ids:
            ids = (ctypes.c_int64 * len(device_ids))(*device_ids)
            rc = lib.axon_start_nrt_profile(ids, len(device_ids))
        else:
            rc = lib.axon_start_nrt_profile(None, 0)
        if rc != 0:
            raise RuntimeError(f"axon_start_nrt_profile rc={rc}")
        try:
            yield
        finally:
            n = lib.axon_stop_nrt_profile(str(output_dir).encode())
            print(f"ntff profile: {n} file(s) written to {output_dir}")

    holder = [_hook]
    mod = types.ModuleType("antenv.axon_hooks")
    mod.set_axon_ntff_profile_hook = lambda h: holder.__setitem__(0, h)
    mod.get_axon_ntff_profile_hook = lambda: holder[0]
    sys.modules["antenv.axon_hooks"] = mod
    import antenv

    antenv.axon_hooks = mod
    return True


QB = 512  # q block width
KC = 128  # kv chunk rows
CG = 3  # chunks per exp group


def _blocks_for(mq):
    """Split mq q-columns into even-width blocks of <=QB, all >=256 wide
    (narrow tails would leave LDWEIGHTS exposed against short streams)."""
    blocks = []
    off = 0
    while mq - off > QB:
        blocks.append((off, QB))
        off += QB
    rem = mq - off
    if rem > 0:
        if rem < 256 and blocks:
            # rebalance last full block + tail into two medium blocks
            off0, _ = blocks.pop()
            tot = QB + rem
            w1 = (tot // 2) & ~1
            blocks.append((off0, w1))
            blocks.append((off0 + w1, tot - w1))
        else:
            blocks.append((off, rem))
    return blocks


def _build_program(S, D, hpc, mq, nk, kv_padded):
    """mq/nk: per-segment q/kv sizes AFTER host padding (all even).
    kv_padded[g]: segment g's kv range ends with one zero dummy row, whose
    exp(0)=1 contribution is subtracted from the softmax sums.
    Outputs O^T [hpc, D, Sq] (unnormalized) and softmax sums [hpc, Sq]; the
    host divides and transposes back."""
    import concourse.bacc as bacc
    import concourse.mybir as mybir
    import concourse.tile as tile

    f32 = mybir.dt.float32
    f32r = mybir.dt.float32r
    bf16 = mybir.dt.bfloat16
    Exp = mybir.ActivationFunctionType.Exp
    scale = 1.0 / float(np.sqrt(D))

    Sq = sum(mq)
    Skv = sum(nk)

    nc = bacc.Bacc("TRN2", target_bir_lowering=False, debug=False)

    qT_d = nc.dram_tensor("qT", [hpc, D, Sq], f32r, kind="ExternalInput")
    kT_d = nc.dram_tensor("kT", [hpc, D, Skv], f32r, kind="ExternalInput")
    v_d = nc.dram_tensor("v", [hpc, Skv, D], bf16, kind="ExternalInput")
    o_d = nc.dram_tensor("o", [hpc, D, Sq], f32, kind="ExternalOutput")
    sums_d = nc.dram_tensor("sums", [hpc, Sq], f32, kind="ExternalOutput")

    seg_q = [(0, mq[0]), (mq[0], mq[0] + mq[1])]
    seg_kv = [(0, nk[0]), (nk[0], nk[0] + nk[1])]

    with tile.TileContext(nc) as tc:
        ctxs = []

        def pool(**kw):
            p = tc.tile_pool(**kw)
            ctxs.append(p)
            return p.__enter__()

        singles = pool(name="singles", bufs=1)
        pt_pool = pool(name="pt", bufs=6)
        acc_pool = pool(name="acc", bufs=3)
        otsb_pool = pool(name="otsb", bufs=3)
        sums_sb_pool = pool(name="sums_sb", bufs=3)
        psum_s = pool(name="psum_s", bufs=2, space="PSUM")
        psum_ot = pool(name="psum_ot", bufs=2, space="PSUM")

        ones_stage = singles.tile([128, 2], f32)
        nc.vector.memset(ones_stage, 1.0)
        ones_bf = singles.tile([128, 2], bf16)
        nc.vector.tensor_copy(ones_bf, ones_stage)

        # bf16 copies of the first q block / all seg0 kv chunks of head 0:
        # half the critical-path DMA bytes, so compute starts earlier. The
        # whole first block of (h0, g0) runs its scores on these tiles.
        FQ = min(512, mq[0])
        FK = min(2048, nk[0])
        qT0_bf = singles.tile([128, FQ], bf16, tag="qT0bf", name="qT0_bf")
        kT0_bf = singles.tile([128, FK], bf16, tag="kT0bf", name="kT0_bf")
        qT0_d = nc.dram_tensor("qT0", [D, FQ], bf16, kind="ExternalInput")
        kT0_d = nc.dram_tensor("kT0", [D, FK], bf16, kind="ExternalInput")

        # ---- input SBUF residency ----
        qT_sb = {}
        kT_sb = {}
        v_sb = {}  # (head, seg) -> [128, C_g, 128] bf16, kv rows packed per seg
        for h in range(hpc):
            qT_sb[h] = singles.tile([128, Sq], f32r, tag=f"qT{h}", name=f"qT_sb{h}")
            kT_sb[h] = singles.tile([128, Skv], f32r, tag=f"kT{h}", name=f"kT_sb{h}")
            for g, (kv0, kv1) in enumerate(seg_kv):
                L = kv1 - kv0
                if L <= 0:
                    continue
                C = (L + KC - 1) // KC
                v_sb[(h, g)] = singles.tile(
                    [128, C, 128], bf16, tag=f"v{h}_{g}", name=f"v_sb{h}_{g}"
                )

        def load_q(h, c0, c1):
            for po in range(c0, c1, 2048):
                pe = min(po + 2048, c1)
                nc.sync.dma_start(out=qT_sb[h][:, po:pe], in_=qT_d[h, :, po:pe])

        def load_k(h, c0, c1):
            for po in range(c0, c1, 2048):
                pe = min(po + 2048, c1)
                nc.sync.dma_start(out=kT_sb[h][:, po:pe], in_=kT_d[h, :, po:pe])

        def load_v(h, g, c0, c1):
            # load v chunks [c0, c1) of segment g (chunk = KC kv rows)
            kv0, kv1 = seg_kv[g]
            L = kv1 - kv0
            vt = v_sb[(h, g)]
            nfull = L // KC
            a, b = min(c0, nfull), min(c1, nfull)
            if b > a:
                src = v_d[h, kv0 + a * KC : kv0 + b * KC, :].rearrange(
                    "(c p) d -> p c d", p=KC
                )
                nc.sync.dma_start(out=vt[:, a:b, :], in_=src)
            rtail = L - nfull * KC
            if rtail and c1 > nfull:
                nc.sync.dma_start(
                    out=vt[:rtail, nfull, :], in_=v_d[h, kv0 + nfull * KC : kv1, :]
                )

        # critical path first: item0 needs qT[h0][:512], kT[h0][g0][:384],
        # then v chunks for the first PVs; everything else streams behind.
        C0 = (nk[0] + KC - 1) // KC
        C1 = (nk[1] + KC - 1) // KC
        # interleave first-head loads by exact consumption order: items 0-1
        # run on the small bf16 fast-start tiles; item 2+ needs f32r qT block0
        # and kv chunks 6+; block1 (item ~6) needs f32r kT cols 0-768 again
        nc.sync.dma_start(out=qT0_bf, in_=qT0_d[:, :])
        po = 0
        for pe_ in (384, 768, 1152, 1536, FK):
            pe_ = min(pe_, FK)
            if pe_ > po:
                nc.sync.dma_start(out=kT0_bf[:, po:pe_], in_=kT0_d[:, po:pe_])
                vc0 = po // KC
                vc1 = min((pe_ + KC - 1) // KC, C0)
                load_v(0, 0, vc0, vc1)
            po = pe_
        # f32r stream for block1 onward (kT cols 0.. needed again from item ~6;
        # qT cols < FQ are only ever read from the bf16 fast tile, skip them)
        load_q(0, FQ, max(1024, FQ))
        load_k(0, seg_kv[0][0], seg_kv[0][1])
        load_v(0, 0, (FK + KC - 1) // KC, C0)
        load_q(0, 1024, 2048)
        load_k(0, seg_kv[1][0], seg_kv[1][1])
        load_v(0, 1, 0, C1)
        load_q(0, 2048, Sq)
        for h in range(1, hpc):
            load_q(h, 0, Sq)
            load_k(h, seg_kv[0][0], seg_kv[0][1])
            load_v(h, 0, 0, C0)
            load_k(h, seg_kv[1][0], seg_kv[1][1])
            load_v(h, 1, 0, C1)



        # ---- build flat work-item list ----
        # item: one (head, seg, block, chunk-group)
        items = []
        for h in range(hpc):
            for g, (q0g, q1g) in enumerate(seg_q):
                kv0, kv1 = seg_kv[g]
                if q1g <= q0g or kv1 <= kv0:
                    continue
                chunks = [
                    (ck, min(KC, kv1 - ck), ci)
                    for ci, ck in enumerate(range(kv0, kv1, KC))
                ]
                # distribute chunks over ceil(C/CG) groups as evenly as
                # possible (balanced FDs keep the ACT pipeline from running
                # dry on a tiny trailing group); the ragged tail chunk, if
                # any, rides in the last group
                ng = (len(chunks) + CG - 1) // CG
                cgroups = []
                off = 0
                for gi in range(ng):
                    take = (len(chunks) - off + (ng - gi - 1)) // (ng - gi)
                    cgroups.append(chunks[off : off + take])
                    off += take
                blocks = _blocks_for(q1g - q0g)
                nchunks = len(chunks)
                for b, (qo, W) in enumerate(blocks):
                    for gi, cg in enumerate(cgroups):
                        items.append(
                            dict(
                                h=h,
                                g=g,
                                q0=q0g + qo,
                                W=W,
                                cg=cg,
                                first=(gi == 0),
                                last=(gi == len(cgroups) - 1),
                                nchunks=nchunks,
                                blk=(h, g, b),
                            )
                        )

        n = len(items)
        state = {}  # per item idx -> (s_ps, pt)
        blk_state = {}  # blk key -> dict(ot=, acc=)
        deferred = {}  # step -> list of callables

        def emitA(i):
            it = items[i]
            h, W, q0, cg = it["h"], it["W"], it["q0"], it["cg"]
            L = len(cg)
            cwmax = max(cw for _, cw, _ in cg)
            s_ps = psum_s.tile([128, CG, QB], f32, tag="s", name="s_ps")
            # the first two chunk-groups of the whole schedule run on the
            # small bf16 fast-start tiles (arrive earliest via DMA)
            fast = (
                it["blk"] == (0, 0, 0)
                and q0 + W <= FQ
                and all(ck + cw <= FK for ck, cw, _ in cg)
            )
            for j, (ck, cw, ci) in enumerate(cg):
                if fast:
                    nc.tensor.matmul(
                        s_ps[:cw, j, :W],
                        kT0_bf[:, ck : ck + cw],
                        qT0_bf[:, q0 : q0 + W],
                        start=True,
                        stop=True,
                    )
                else:
                    nc.tensor.matmul(
                        s_ps[:cw, j, :W],
                        kT_sb[h][:, ck : ck + cw],
                        qT_sb[h][:, q0 : q0 + W],
                        start=True,
                        stop=True,
                    )
            pt = pt_pool.tile([128, CG, QB], bf16, tag="pt", name="pt")
            nc.scalar.activation(
                pt[:cwmax, :L, :W], s_ps[:cwmax, :L, :W], Exp, scale=scale
            )
            state[i] = pt

        def emitB(i):
            it = items[i]
            h, g, W, cg = it["h"], it["g"], it["W"], it["cg"]
            pt = state.pop(i)
            bk = it["blk"]
            if it["first"]:
                blk_state[bk] = dict(
                    ot=psum_ot.tile([128, QB], f32, tag="ot", name="ot_ps"),
                    acc=acc_pool.tile([128, CG, QB], bf16, tag="acc", name="acc"),
                )
            bs = blk_state[bk]
            ot_ps, acc = bs["ot"], bs["acc"]
            nch = it["nchunks"]
            for j, (ck, cw, ci) in enumerate(cg):
                nc.tensor.matmul(
                    ot_ps[:, :W],
                    v_sb[(h, g)][:cw, ci, :],
                    pt[:cw, j, :W],
                    start=(ci == 0),
                    stop=(ci == nch - 1),
                )
            # DVE: accumulate exp chunks for the softmax sums.  Ragged tail
            # chunks (cw < KC) are added separately so the exp of stale PSUM
            # rows never reaches the accumulator.
            L = len(cg)
            nf = sum(1 for _, cw, _ in cg if cw == KC)
            if it["first"] and L == CG and nf == L:
                nc.vector.tensor_copy(acc[:, :, :W], pt[:, :, :W])
            else:
                if it["first"]:
                    nc.vector.memset(acc[:, :, :W], 0.0)
                if nf:
                    nc.vector.tensor_add(
                        acc[:, :nf, :W], acc[:, :nf, :W], pt[:, :nf, :W]
                    )
                for j in range(nf, L):
                    cw = cg[j][1]
                    nc.vector.tensor_add(
                        acc[:cw, j, :W], acc[:cw, j, :W], pt[:cw, j, :W]
                    )
            if it["last"]:
                # flush O^T while collapsing the accumulator
                ot_sb = otsb_pool.tile([128, QB], f32, tag="otsb", name="ot_sb")
                nc.vector.tensor_copy(ot_sb[:, :W], ot_ps[:, :W])
                q0 = it["q0"]
                nc.sync.dma_start(out=o_d[h, :, q0 : q0 + W], in_=ot_sb[:, :W])
                if CG >= 2:
                    nc.vector.tensor_add(
                        acc[:, 0, :W], acc[:, 0, :W], acc[:, 1, :W]
                    )
                if CG >= 3:
                    nc.vector.tensor_add(
                        acc[:, 0, :W], acc[:, 0, :W], acc[:, 2, :W]
                    )
                pad = kv_padded[g]

                def fin(acc=acc, W=W, q0=q0, h=h, pad=pad, ot_ps=ot_ps):
                    # reuse the block's drained ot PSUM bank for the sums
                    # matmul (ot has been copied to SBUF by now)
                    nc.tensor.matmul(
                        ot_ps[:2, :W],
                        ones_bf[:, :2],
                        acc[:, 0, :W],
                        start=True,
                        stop=True,
                    )
                    sums_sb = sums_sb_pool.tile([1, QB], f32, tag="sums_sb", name="sums_sb")
                    nc.vector.tensor_scalar_add(
                        sums_sb[:1, :W],
                        ot_ps[:1, :W],
                        -1.0 if pad else 0.0,
                    )
                    nc.sync.dma_start(
                        out=sums_d[h : h + 1, q0 : q0 + W], in_=sums_sb[:1, :W]
                    )

                # must land at step >= i+3 so emitB(i) has already been
                # emitted (deferred ops run before emitB within a step)
                deferred.setdefault(i + 4 if i + 4 < n else i + 3, []).append(fin)

        for step in range(n + 6):
            if step < n:
                emitA(step)
            # deferred block-finalizers go between the scores (ACT feed) and
            # the PVs so the ones-matmul never delays the next exp
            for fn in deferred.pop(step, []):
                fn()
            if 2 <= step < n + 2:
                emitB(step - 2)

        for p in reversed(ctxs):
            p.__exit__(None, None, None)

    nc.compile()
    return nc


def kernel(q, k, v, q_segment_ids, kv_segment_ids):
    global last_exec_time_ns
    import ml_dtypes
    from concourse.bass_utils import run_bass_kernel_spmd

    q = np.asarray(q, dtype=np.float32)
    k = np.asarray(k, dtype=np.float32)
    v = np.asarray(v, dtype=np.float32)
    q_seg = np.asarray(q_segment_ids, dtype=np.int32)
    kv_seg = np.asarray(kv_segment_ids, dtype=np.int32)

    B, H, S, D = q.shape
    assert B == 1
    ncores = 8
    hpc = H // ncores

    qperm = np.argsort(q_seg[0], kind="stable")
    kvperm = np.argsort(kv_seg[0], kind="stable")
    m0 = int((q_seg[0] == 0).sum())
    n0 = int((kv_seg[0] == 0).sum())
    m1, n1 = S - m0, S - n0

    # fp32r matmuls need even free sizes -> pad every segment to even length
    # (q dummies: computed but never stored; kv dummies: k=0,v=0 rows whose
    # exp(0)=1 is subtracted from the softmax sums on device)
    def pad_seg(arr_s, lens):
        parts, out_lens = [], []
        off = 0
        for L in lens:
            seg = arr_s[:, off : off + L, :]
            if L % 2:
                z = np.zeros((arr_s.shape[0], 1, arr_s.shape[2]), arr_s.dtype)
                seg = np.concatenate([seg, z], axis=1)
            parts.append(seg)
            out_lens.append(seg.shape[1])
            off += L
        return np.concatenate(parts, axis=1), out_lens

    q_s, mq = pad_seg(q[0][:, qperm, :], [m0, m1])
    k_s, nk = pad_seg(k[0][:, kvperm, :], [n0, n1])
    v_s, _ = pad_seg(v[0][:, kvperm, :], [n0, n1])
    kv_padded = (n0 % 2 == 1, n1 % 2 == 1)
    qT = np.ascontiguousarray(np.swapaxes(q_s, 1, 2))  # [H, D, Sq]
    kT = np.ascontiguousarray(np.swapaxes(k_s, 1, 2))
    v_bf = v_s.astype(ml_dtypes.bfloat16)

    key = (S, D, hpc, tuple(mq), tuple(nk), kv_padded)
    if key not in _PROGRAM_CACHE:
        _PROGRAM_CACHE.clear()
        _PROGRAM_CACHE[key] = _build_program(S, D, hpc, mq, nk, kv_padded)
    nc = _PROGRAM_CACHE[key]

    FQ = min(512, mq[0])
    FK = min(2048, nk[0])
    in_maps = []
    for i in range(ncores):
        hs = slice(i * hpc, (i + 1) * hpc)
        in_maps.append(
            {
                "qT": np.ascontiguousarray(qT[hs]),
                "kT": np.ascontiguousarray(kT[hs]),
                "v": np.ascontiguousarray(v_bf[hs]),
                "qT0": np.ascontiguousarray(qT[i * hpc, :, :FQ]).astype(
                    ml_dtypes.bfloat16
                ),
                "kT0": np.ascontiguousarray(kT[i * hpc, :, :FK]).astype(
                    ml_dtypes.bfloat16
                ),
            }
        )

    trace = bool(int(os.environ.get("KERNEL_TRACE", "0")))
    tmpdir = None
    if trace:
        trace = _install_ntff_hook()
        tmpdir = os.environ.get("KERNEL_TRACE_DIR") or None
        if trace:
            import concourse.bass_utils as _bu

            _bu.upload_artifacts = lambda d: d  # no bucket access here
    res = run_bass_kernel_spmd(
        nc, in_maps, core_ids=list(range(ncores)), trace=trace, tmpdir=tmpdir
    )
    last_exec_time_ns = res.exec_time_ns

    oT_pad = np.concatenate([res.results[i]["o"] for i in range(ncores)], axis=0)
    sums_pad = np.concatenate(
        [res.results[i]["sums"] for i in range(ncores)], axis=0
    )
    # normalize (device returns unnormalized O^T and softmax sums),
    # transpose back to [H, Sq, D]
    o_pad = np.swapaxes(oT_pad / sums_pad[:, None, :], 1, 2)
    # drop q dummy rows (end of each padded segment), then unsort
    o_sorted = np.concatenate([o_pad[:, :m0, :], o_pad[:, mq[0] : mq[0] + m1, :]], 1)
    out = np.empty((H, S, D), dtype=np.float32)
    out[:, qperm, :] = o_sorted
    return np.ascontiguousarray(out[None], dtype=np.float32)


# revision 32
# speedup vs baseline: 1.0250x; 1.0250x over previous
"""Segment-masked attention kernel for Trainium2 (8 NeuronCores).

Problem: B=1, H=16, S=4096, D=128, NSEG=2 segment-id masked softmax attention.

Strategy (v2):
  * Host: stable-argsort q/kv positions by segment id -> two dense
    block-diagonal attentions (one per segment), ~half the FLOPs, no masks.
    Outputs are scattered back to original q order on host.
  * Shard: 2 heads per core across 8 cores (head-parallel, no comms).
  * Per (head, segment): q processed in blocks of <=512 columns (tail
    rebalanced so every block is >=256 wide); kv in chunks of 128 rows,
    chunks grouped in threes so each ACT(exp) instruction covers
    FD=3*512=1536 elements (amortizes the ~300-cycle ACT issue overhead).
  * Dtypes: scores matmul in fp32r (1 cycle/col on PE); exp output pt in
    bf16; PV matmul bf16xbf16 (1 cycle/col).
  * Softmax sums WITHOUT the per-chunk ones-matmul (which would cost as many
    PE column-cycles as the whole PV): DVE accumulates pt chunk-wise into a
    wide bf16 accumulator (tensor_add at 2x rate), collapses it at block end,
    and a single 1-row ones-matmul per block (N=W columns) finishes the
    partition-dim reduction.  PE sums cost drops ~16x.
  * Software pipeline depth 2 over (block, chunk-group) items keeps PE
    streaming while ACT computes exp: scores(i) || exp(i-1) || pv(i-2).
  * kv segments are padded to even length with zero rows whose exp(0)=1 is
    subtracted from the sums; no max-subtraction is needed (scaled scores are
    ~N(0,1), exp never overflows fp32, softmax is shift invariant).
"""

import os

import numpy as np

_PROGRAM_CACHE = {}
last_exec_time_ns = None


def _install_ntff_hook():
    """Provide antenv.axon_hooks (missing in this image) so that
    run_bass_kernel_spmd(trace=True) can capture an NTFF profile."""
    import contextlib
    import ctypes
    import sys
    import types

    try:
        from antenv.axon_hooks import get_axon_ntff_profile_hook  # noqa: F401

        return True  # real module exists
    except ImportError:
        pass

    so_path = "/opt/axon/libaxon_pjrt.so"
    if not os.path.exists(so_path):
        return False
    lib = ctypes.CDLL(so_path)
    if not hasattr(lib, "axon_start_nrt_profile"):
        return False
    lib.axon_start_nrt_profile.argtypes = [
        ctypes.POINTER(ctypes.c_int64),
        ctypes.c_size_t,
    ]
    lib.axon_start_nrt_profile.restype = ctypes.c_int64
    lib.axon_stop_nrt_profile.argtypes = [ctypes.c_char_p]
    lib.axon_stop_nrt_profile.restype = ctypes.c_int64

    @contextlib.contextmanager
    def _hook(output_dir, device_ids):
        import jax

        jax.devices()
        if device_ids:
            ids = (ctypes.c_int64 * len(device_ids))(*device_ids)
            rc = lib.axon_start_nrt_profile(ids, len(device_ids))
        else:
            rc = lib.axon_start_nrt_profile(None, 0)
        if rc != 0:
            raise RuntimeError(f"axon_start_nrt_profile rc={rc}")
        try:
            yield
        finally:
            n = lib.axon_stop_nrt_profile(str(output_dir).encode())
            print(f"ntff profile: {n} file(s) written to {output_dir}")

    holder = [_hook]
    mod = types.ModuleType("antenv.axon_hooks")
    mod.set_axon_ntff_profile_hook = lambda h: holder.__setitem__(0, h)
    mod.get_axon_ntff_profile_hook = lambda: holder[0]
    sys.modules["antenv.axon_hooks"] = mod
    import antenv

    antenv.axon_hooks = mod
    return True


QB = 512  # q block width
KC = 128  # kv chunk rows
CG = 3  # chunks per exp group


def _blocks_for(mq):
    """Split mq q-columns into even-width blocks of <=QB, all >=256 wide
    (narrow tails would leave LDWEIGHTS exposed against short streams)."""
    blocks = []
    off = 0
    while mq - off > QB:
        blocks.append((off, QB))
        off += QB
    rem = mq - off
    if rem > 0:
        if rem < 256 and blocks:
            # rebalance last full block + tail into two medium blocks
            off0, _ = blocks.pop()
            tot = QB + rem
            w1 = (tot // 2) & ~1
            blocks.append((off0, w1))
            blocks.append((off0 + w1, tot - w1))
        else:
            blocks.append((off, rem))
    return blocks


def _build_program(S, D, hpc, mq, nk, kv_padded):
    """mq/nk: per-segment q/kv sizes AFTER host padding (all even).
    kv_padded[g]: segment g's kv range ends with one zero dummy row, whose
    exp(0)=1 contribution is subtracted from the softmax sums.
    Outputs O^T [hpc, D, Sq] (unnormalized) and softmax sums [hpc, Sq]; the
    host divides and transposes back."""
    import concourse.bacc as bacc
    import concourse.mybir as mybir
    import concourse.tile as tile

    f32 = mybir.dt.float32
    f32r = mybir.dt.float32r
    bf16 = mybir.dt.bfloat16
    Exp = mybir.ActivationFunctionType.Exp
    scale = 1.0 / float(np.sqrt(D))

    Sq = sum(mq)
    Skv = sum(nk)

    nc = bacc.Bacc("TRN2", target_bir_lowering=False, debug=False)

    qT_d = nc.dram_tensor("qT", [hpc, D, Sq], f32r, kind="ExternalInput")
    kT_d = nc.dram_tensor("kT", [hpc, D, Skv], f32r, kind="ExternalInput")
    v_d = nc.dram_tensor("v", [hpc, Skv, D], bf16, kind="ExternalInput")
    o_d = nc.dram_tensor("o", [hpc, D, Sq], f32, kind="ExternalOutput")
    sums_d = nc.dram_tensor("sums", [hpc, Sq], f32, kind="ExternalOutput")

    seg_q = [(0, mq[0]), (mq[0], mq[0] + mq[1])]
    seg_kv = [(0, nk[0]), (nk[0], nk[0] + nk[1])]

    with tile.TileContext(nc) as tc:
        ctxs = []

        def pool(**kw):
            p = tc.tile_pool(**kw)
            ctxs.append(p)
            return p.__enter__()

        singles = pool(name="singles", bufs=1)
        pt_pool = pool(name="pt", bufs=6)
        acc_pool = pool(name="acc", bufs=3)
        otsb_pool = pool(name="otsb", bufs=3)
        sums_sb_pool = pool(name="sums_sb", bufs=3)
        psum_s = pool(name="psum_s", bufs=2, space="PSUM")
        psum_ot = pool(name="psum_ot", bufs=2, space="PSUM")

        ones_stage = singles.tile([128, 2], f32)
        nc.vector.memset(ones_stage, 1.0)
        ones_bf = singles.tile([128, 2], bf16)
        nc.vector.tensor_copy(ones_bf, ones_stage)

        # bf16 copies of the first q block / all seg0 kv chunks of head 0:
        # half the critical-path DMA bytes, so compute starts earlier. The
        # whole first block of (h0, g0) runs its scores on these tiles.
        FQ = min(512, mq[0])
        FK = min(768, nk[0])
        qT0_bf = singles.tile([128, FQ], bf16, tag="qT0bf", name="qT0_bf")
        kT0_bf = singles.tile([128, FK], bf16, tag="kT0bf", name="kT0_bf")
        qT0_d = nc.dram_tensor("qT0", [D, FQ], bf16, kind="ExternalInput")
        kT0_d = nc.dram_tensor("kT0", [D, FK], bf16, kind="ExternalInput")

        # ---- input SBUF residency ----
        qT_sb = {}
        kT_sb = {}
        v_sb = {}  # (head, seg) -> [128, C_g, 128] bf16, kv rows packed per seg
        for h in range(hpc):
            qT_sb[h] = singles.tile([128, Sq], f32r, tag=f"qT{h}", name=f"qT_sb{h}")
            kT_sb[h] = singles.tile([128, Skv], f32r, tag=f"kT{h}", name=f"kT_sb{h}")
            for g, (kv0, kv1) in enumerate(seg_kv):
                L = kv1 - kv0
                if L <= 0:
                    continue
                C = (L + KC - 1) // KC
                v_sb[(h, g)] = singles.tile(
                    [128, C, 128], bf16, tag=f"v{h}_{g}", name=f"v_sb{h}_{g}"
                )

        def load_q(h, c0, c1):
            for po in range(c0, c1, 2048):
                pe = min(po + 2048, c1)
                nc.sync.dma_start(out=qT_sb[h][:, po:pe], in_=qT_d[h, :, po:pe])

        def load_k(h, c0, c1):
            for po in range(c0, c1, 2048):
                pe = min(po + 2048, c1)
                nc.sync.dma_start(out=kT_sb[h][:, po:pe], in_=kT_d[h, :, po:pe])

        def load_v(h, g, c0, c1):
            # load v chunks [c0, c1) of segment g (chunk = KC kv rows)
            kv0, kv1 = seg_kv[g]
            L = kv1 - kv0
            vt = v_sb[(h, g)]
            nfull = L // KC
            a, b = min(c0, nfull), min(c1, nfull)
            if b > a:
                src = v_d[h, kv0 + a * KC : kv0 + b * KC, :].rearrange(
                    "(c p) d -> p c d", p=KC
                )
                nc.sync.dma_start(out=vt[:, a:b, :], in_=src)
            rtail = L - nfull * KC
            if rtail and c1 > nfull:
                nc.sync.dma_start(
                    out=vt[:rtail, nfull, :], in_=v_d[h, kv0 + nfull * KC : kv1, :]
                )

        # critical path first: item0 needs qT[h0][:512], kT[h0][g0][:384],
        # then v chunks for the first PVs; everything else streams behind.
        C0 = (nk[0] + KC - 1) // KC
        C1 = (nk[1] + KC - 1) // KC
        # interleave first-head loads by exact consumption order: items 0-1
        # run on the small bf16 fast-start tiles; item 2+ needs f32r qT block0
        # and kv chunks 6+; block1 (item ~6) needs f32r kT cols 0-768 again
        k0 = seg_kv[0][0]
        nc.sync.dma_start(out=qT0_bf, in_=qT0_d[:, :])
        nc.sync.dma_start(out=kT0_bf[:, :384], in_=kT0_d[:, :384])
        load_v(0, 0, 0, 3)
        if FK > 384:
            nc.sync.dma_start(out=kT0_bf[:, 384:FK], in_=kT0_d[:, 384:FK])
        load_v(0, 0, 3, 6)
        load_q(0, 0, 512)
        load_k(0, k0 + 768, k0 + 1152)
        load_v(0, 0, 6, 9)
        load_k(0, k0 + 1152, k0 + 1536)
        load_v(0, 0, 9, 12)
        load_k(0, k0 + 1536, seg_kv[0][1])
        load_v(0, 0, 12, C0)
        load_k(0, k0, k0 + 768)
        load_q(0, 512, 1024)
        load_q(0, 1024, 2048)
        load_k(0, seg_kv[1][0], seg_kv[1][1])
        load_v(0, 1, 0, C1)
        load_q(0, 2048, Sq)
        for h in range(1, hpc):
            load_q(h, 0, Sq)
            load_k(h, seg_kv[0][0], seg_kv[0][1])
            load_v(h, 0, 0, C0)
            load_k(h, seg_kv[1][0], seg_kv[1][1])
            load_v(h, 1, 0, C1)



        # ---- build flat work-item list ----
        # item: one (head, seg, block, chunk-group)
        items = []
        for h in range(hpc):
            for g, (q0g, q1g) in enumerate(seg_q):
                kv0, kv1 = seg_kv[g]
                if q1g <= q0g or kv1 <= kv0:
                    continue
                chunks = [
                    (ck, min(KC, kv1 - ck), ci)
                    for ci, ck in enumerate(range(kv0, kv1, KC))
                ]
                # distribute chunks over ceil(C/CG) groups as evenly as
                # possible (balanced FDs keep the ACT pipeline from running
                # dry on a tiny trailing group); the ragged tail chunk, if
                # any, rides in the last group
                ng = (len(chunks) + CG - 1) // CG
                cgroups = []
                off = 0
                for gi in range(ng):
                    take = (len(chunks) - off + (ng - gi - 1)) // (ng - gi)
                    cgroups.append(chunks[off : off + take])
                    off += take
                blocks = _blocks_for(q1g - q0g)
                nchunks = len(chunks)
                for b, (qo, W) in enumerate(blocks):
                    for gi, cg in enumerate(cgroups):
                        items.append(
                            dict(
                                h=h,
                                g=g,
                                q0=q0g + qo,
                                W=W,
                                cg=cg,
                                first=(gi == 0),
                                last=(gi == len(cgroups) - 1),
                                nchunks=nchunks,
                                blk=(h, g, b),
                            )
                        )

        n = len(items)
        state = {}  # per item idx -> (s_ps, pt)
        blk_state = {}  # blk key -> dict(ot=, acc=)
        deferred = {}  # step -> list of callables

        def emitA(i):
            it = items[i]
            h, W, q0, cg = it["h"], it["W"], it["q0"], it["cg"]
            L = len(cg)
            cwmax = max(cw for _, cw, _ in cg)
            s_ps = psum_s.tile([128, CG, QB], f32, tag="s", name="s_ps")
            # the first two chunk-groups of the whole schedule run on the
            # small bf16 fast-start tiles (arrive earliest via DMA)
            fast = (
                it["blk"] == (0, 0, 0)
                and q0 + W <= FQ
                and all(ck + cw <= FK for ck, cw, _ in cg)
            )
            for j, (ck, cw, ci) in enumerate(cg):
                if fast:
                    nc.tensor.matmul(
                        s_ps[:cw, j, :W],
                        kT0_bf[:, ck : ck + cw],
                        qT0_bf[:, q0 : q0 + W],
                        start=True,
                        stop=True,
                    )
                else:
                    nc.tensor.matmul(
                        s_ps[:cw, j, :W],
                        kT_sb[h][:, ck : ck + cw],
                        qT_sb[h][:, q0 : q0 + W],
                        start=True,
                        stop=True,
                    )
            pt = pt_pool.tile([128, CG, QB], bf16, tag="pt", name="pt")
            nc.scalar.activation(
                pt[:cwmax, :L, :W], s_ps[:cwmax, :L, :W], Exp, scale=scale
            )
            state[i] = pt

        def emitB(i):
            it = items[i]
            h, g, W, cg = it["h"], it["g"], it["W"], it["cg"]
            pt = state.pop(i)
            bk = it["blk"]
            if it["first"]:
                blk_state[bk] = dict(
                    ot=psum_ot.tile([128, QB], f32, tag="ot", name="ot_ps"),
                    acc=acc_pool.tile([128, CG, QB], bf16, tag="acc", name="acc"),
                )
            bs = blk_state[bk]
            ot_ps, acc = bs["ot"], bs["acc"]
            nch = it["nchunks"]
            for j, (ck, cw, ci) in enumerate(cg):
                nc.tensor.matmul(
                    ot_ps[:, :W],
                    v_sb[(h, g)][:cw, ci, :],
                    pt[:cw, j, :W],
                    start=(ci == 0),
                    stop=(ci == nch - 1),
                )
            # DVE: accumulate exp chunks for the softmax sums.  Ragged tail
            # chunks (cw < KC) are added separately so the exp of stale PSUM
            # rows never reaches the accumulator.
            L = len(cg)
            nf = sum(1 for _, cw, _ in cg if cw == KC)
            if it["first"] and L == CG and nf == L:
                nc.vector.tensor_copy(acc[:, :, :W], pt[:, :, :W])
            else:
                if it["first"]:
                    nc.vector.memset(acc[:, :, :W], 0.0)
                if nf:
                    nc.vector.tensor_add(
                        acc[:, :nf, :W], acc[:, :nf, :W], pt[:, :nf, :W]
                    )
                for j in range(nf, L):
                    cw = cg[j][1]
                    nc.vector.tensor_add(
                        acc[:cw, j, :W], acc[:cw, j, :W], pt[:cw, j, :W]
                    )
            if it["last"]:
                # flush O^T while collapsing the accumulator
                ot_sb = otsb_pool.tile([128, QB], f32, tag="otsb", name="ot_sb")
                nc.vector.tensor_copy(ot_sb[:, :W], ot_ps[:, :W])
                q0 = it["q0"]
                nc.sync.dma_start(out=o_d[h, :, q0 : q0 + W], in_=ot_sb[:, :W])
                if CG >= 2:
                    nc.vector.tensor_add(
                        acc[:, 0, :W], acc[:, 0, :W], acc[:, 1, :W]
                    )
                if CG >= 3:
                    nc.vector.tensor_add(
                        acc[:, 0, :W], acc[:, 0, :W], acc[:, 2, :W]
                    )
                pad = kv_padded[g]

                def fin(acc=acc, W=W, q0=q0, h=h, pad=pad, ot_ps=ot_ps):
                    # reuse the block's drained ot PSUM bank for the sums
                    # matmul (ot has been copied to SBUF by now)
                    nc.tensor.matmul(
                        ot_ps[:2, :W],
                        ones_bf[:, :2],
                        acc[:, 0, :W],
                        start=True,
                        stop=True,
                    )
                    sums_sb = sums_sb_pool.tile([1, QB], f32, tag="sums_sb", name="sums_sb")
                    nc.vector.tensor_scalar_add(
                        sums_sb[:1, :W],
                        ot_ps[:1, :W],
                        -1.0 if pad else 0.0,
                    )
                    nc.sync.dma_start(
                        out=sums_d[h : h + 1, q0 : q0 + W], in_=sums_sb[:1, :W]
                    )

                # must land at step >= i+3 so emitB(i) has already been
                # emitted (deferred ops run before emitB within a step)
                deferred.setdefault(i + 4 if i + 4 < n else i + 3, []).append(fin)

        for step in range(n + 6):
            if step < n:
                emitA(step)
            # deferred block-finalizers go between the scores (ACT feed) and
            # the PVs so the ones-matmul never delays the next exp
            for fn in deferred.pop(step, []):
                fn()
            if 2 <= step < n + 2:
                emitB(step - 2)

        for p in reversed(ctxs):
            p.__exit__(None, None, None)

    nc.compile()
    return nc


def kernel(q, k, v, q_segment_ids, kv_segment_ids):
    global last_exec_time_ns
    import ml_dtypes
    from concourse.bass_utils import run_bass_kernel_spmd

    q = np.asarray(q, dtype=np.float32)
    k = np.asarray(k, dtype=np.float32)
    v = np.asarray(v, dtype=np.float32)
    q_seg = np.asarray(q_segment_ids, dtype=np.int32)
    kv_seg = np.asarray(kv_segment_ids, dtype=np.int32)

    B, H, S, D = q.shape
    assert B == 1
    ncores = 8
    hpc = H // ncores

    qperm = np.argsort(q_seg[0], kind="stable")
    kvperm = np.argsort(kv_seg[0], kind="stable")
    m0 = int((q_seg[0] == 0).sum())
    n0 = int((kv_seg[0] == 0).sum())
    m1, n1 = S - m0, S - n0

    # fp32r matmuls need even free sizes -> pad every segment to even length
    # (q dummies: computed but never stored; kv dummies: k=0,v=0 rows whose
    # exp(0)=1 is subtracted from the softmax sums on device)
    def pad_seg(arr_s, lens):
        parts, out_lens = [], []
        off = 0
        for L in lens:
            seg = arr_s[:, off : off + L, :]
            if L % 2:
                z = np.zeros((arr_s.shape[0], 1, arr_s.shape[2]), arr_s.dtype)
                seg = np.concatenate([seg, z], axis=1)
            parts.append(seg)
            out_lens.append(seg.shape[1])
            off += L
        return np.concatenate(parts, axis=1), out_lens

    q_s, mq = pad_seg(q[0][:, qperm, :], [m0, m1])
    k_s, nk = pad_seg(k[0][:, kvperm, :], [n0, n1])
    v_s, _ = pad_seg(v[0][:, kvperm, :], [n0, n1])
    kv_padded = (n0 % 2 == 1, n1 % 2 == 1)
    qT = np.ascontiguousarray(np.swapaxes(q_s, 1, 2))  # [H, D, Sq]
    kT = np.ascontiguousarray(np.swapaxes(k_s, 1, 2))
    v_bf = v_s.astype(ml_dtypes.bfloat16)

    key = (S, D, hpc, tuple(mq), tuple(nk), kv_padded)
    if key not in _PROGRAM_CACHE:
        _PROGRAM_CACHE.clear()
        _PROGRAM_CACHE[key] = _build_program(S, D, hpc, mq, nk, kv_padded)
    nc = _PROGRAM_CACHE[key]

    FQ = min(512, mq[0])
    FK = min(768, nk[0])
    in_maps = []
    for i in range(ncores):
        hs = slice(i * hpc, (i + 1) * hpc)
        in_maps.append(
            {
                "qT": np.ascontiguousarray(qT[hs]),
                "kT": np.ascontiguousarray(kT[hs]),
                "v": np.ascontiguousarray(v_bf[hs]),
                "qT0": np.ascontiguousarray(qT[i * hpc, :, :FQ]).astype(
                    ml_dtypes.bfloat16
                ),
                "kT0": np.ascontiguousarray(kT[i * hpc, :, :FK]).astype(
                    ml_dtypes.bfloat16
                ),
            }
        )

    trace = bool(int(os.environ.get("KERNEL_TRACE", "0")))
    tmpdir = None
    if trace:
        trace = _install_ntff_hook()
        tmpdir = os.environ.get("KERNEL_TRACE_DIR") or None
        if trace:
            import concourse.bass_utils as _bu

            _bu.upload_artifacts = lambda d: d  # no bucket access here
    res = run_bass_kernel_spmd(
        nc, in_maps, core_ids=list(range(ncores)), trace=trace, tmpdir=tmpdir
    )
    last_exec_time_ns = res.exec_time_ns

    oT_pad = np.concatenate([res.results[i]["o"] for i in range(ncores)], axis=0)
    sums_pad = np.concatenate(
        [res.results[i]["sums"] for i in range(ncores)], axis=0
    )
    # normalize (device returns unnormalized O^T and softmax sums),
    # transpose back to [H, Sq, D]
    o_pad = np.swapaxes(oT_pad / sums_pad[:, None, :], 1, 2)
    # drop q dummy rows (end of each padded segment), then unsort
    o_sorted = np.concatenate([o_pad[:, :m0, :], o_pad[:, mq[0] : mq[0] + m1, :]], 1)
    out = np.empty((H, S, D), dtype=np.float32)
    out[:, qperm, :] = o_sorted
    return np.ascontiguousarray(out[None], dtype=np.float32)
